# revision 20
# baseline (speedup 1.0000x reference)
"""DeepEMD Trainium2 kernel: batched 49x49 entropic-OT (Sinkhorn) similarity.

Strategy (8 NeuronCores, data-parallel over batch):
- Each core gets 128 batches. Host prepacks, per (chunk j of 128 channels,
  batch b), a matrix [Ahi | Alo] (128 x 198) where A = [Q | P | 1] (99 cols)
  split into bf16 hi + bf16 lo (near-lossless fp32 split), laid out so DMA
  loads are large contiguous runs.
- PE computes, per batch, [S | X] = Ahi^T [Ahi | Alo] (99 x 198, fp32 PSUM)
  with ONE accumulating bf16 matmul per channel chunk (weights widened to
  128 cols to engage fast-weight-load; junk rows ignored). The true Gram is
  G = S + X + X^T (the lo.lo term ~1e-5 is dropped); the X^T merge happens
  later as strided-view adds in the flat domain.
- A per-batch SBUF->SBUF DMA flattens [99, 198] into row b of a
  [128, 99*198] tile: everything after that runs batch-on-partitions with
  full 128-lane DVE. G contains Q^T P, P^T Q, column sums (ones row) and
  diag blocks -> similarity map, norms and weight vectors are cheap fixups.
- Sinkhorn runs in the *linear* domain (K = exp((sim-1)/eps + 16)) with
  Gauss-Seidel updates us = r/(K vs), vs = c/(K^T us). The reference's 100
  log-domain iterations are converged to ~1e-12 by 20; ITERS linear f32
  iterations reach ~2.5e-4 at 6 / ~1e-5 at 10.
- logits[b] = T * sum(flow * sim) = T * us^T ((K.sim) vs), two big DVE ops.
"""

import os
import sys

import numpy as np

sys.path.insert(0, "/opt/trn_rl_repo")

import concourse.bass as bass
import concourse.bacc as bacc
import concourse.mybir as mybir
from concourse import tile
from concourse.bass_utils import run_bass_kernel_spmd

import ml_dtypes

B_FULL, C, HW = 1024, 512, 49
NCORE = 8
BS = B_FULL // NCORE  # 128 batches per core
NCH = C // 128  # 4 chunks of 128 channels (PE contraction dim)
AC = 2 * HW + 1  # 99 augmented columns [Q | P | 1]
AC2 = 2 * AC  # 198 = [hi | lo]
GRP = 8  # batches per DMA group
NGRP = BS // GRP
ITERS = 6
EPS_S = 0.05
TEMP = 12.5 / HW
EXP_BIAS = -4.0  # exp((sim-1)/eps) * e^16 rescale; cancels in us*K*vs

f32 = mybir.dt.float32
bf16 = mybir.dt.bfloat16
Alu = mybir.AluOpType
Act = mybir.ActivationFunctionType
AxX = mybir.AxisListType.X


def build_nc(debug=False):
    nc = bacc.Bacc(None, target_bir_lowering=False, debug=debug)
    aug = nc.declare_dram_parameter(
        "aug", [NGRP, 128, NCH * GRP * AC2], bf16, isOutput=False
    )
    outp = nc.declare_dram_parameter("out", [BS, 1], f32, isOutput=True)

    FW = AC * AC2  # 19602 flat row width

    with tile.TileContext(nc) as tc:
        with (
            tc.tile_pool(name="big", bufs=1) as big,
            tc.tile_pool(name="stage", bufs=2) as stg,
            tc.tile_pool(name="gcopy", bufs=8) as gcp,
            tc.tile_pool(name="work", bufs=5) as wrk,
            tc.tile_pool(name="small", bufs=1) as sml,
            tc.tile_pool(name="psum", bufs=8, space="PSUM") as pp,
        ):
            flatG = big.tile([BS, FW], f32, tag="flatG", name="flatG")

            # ---------------- Phase 1: DMA in + Gram + flatten ----------------
            JW = GRP * AC2  # cols per chunk-slab
            for g in range(NGRP):
                th = stg.tile([128, NCH * JW], bf16, tag="h", name="hg")
                # host pre-sequenced: one fully-contiguous DRAM span per group
                nc.sync.dma_start(th[:], aug[g, :, :])
                for bb in range(GRP):
                    b = g * GRP + bb
                    ps = pp.tile([128, AC2], f32, tag="gram", name="gram")
                    for j in range(NCH):
                        base = j * JW + bb * AC2
                        # stationary: hi cols widened to 128 (spills into own
                        # lo region -> junk G rows 99..127, never read)
                        nc.tensor.matmul(
                            ps[:],
                            th[:, base : base + 128],
                            th[:, base : base + AC2],
                            start=(j == 0),
                            stop=(j == NCH - 1),
                        )
                    gs = gcp.tile([AC, AC2], f32, tag="gs", name="gs")
                    nc.vector.tensor_copy(gs[:], ps[0:AC, :])
                    # flatten [99, 198] -> one batch-major row; alternate the
                    # issuing HWDGE queue so neither sequencer becomes the
                    # phase-1 pacer
                    dmae = nc.sync if (b % 2 == 0) else nc.scalar
                    dmae.dma_start(flatG[b : b + 1, :], gs[:])

            # ---------------- Phase 1.5: fixup to sim/K/marginals -------------
            # flat row layout: element (m, n) of S at m*198+n, of X at
            # m*198+99+n, where S = hi^T hi, X = hi^T lo, G = S + X + X^T.
            fA = flatG[:]

            def mkview(row0, col0, dims):
                v = fA[:, row0 * AC2 + col0 : row0 * AC2 + col0 + 1].copy()
                v.ap = bass.mybir.VecI64Pair([list(v.ap[0])] + dims)
                return v

            def blk(row0, col0):
                # [128, 49, 49] strided view: (m, n) -> (row0+m)*198 + col0+n
                return mkview(row0, col0, [[AC2, HW], [1, HW]])

            def blkT(row0, col0):
                # transposed: out (m, n) reads (row0+n)*198 + col0+m
                return mkview(row0, col0, [[1, HW], [AC2, HW]])

            def dview(row0, col0):
                # [128, 49] diagonal view: (row0+m)*198 + col0+m, stride 199
                return mkview(row0, col0, [[AC2 + 1, HW]])

            Sqp, Xqp, XpqT = blk(0, HW), blk(0, AC + HW), blkT(HW, AC)
            Spq, Xpq, XqpT = blk(HW, 0), blk(HW, AC), blkT(0, AC + HW)
            dqS, dqX = dview(0, 0), dview(0, AC)
            dpS, dpX = dview(HW, HW), dview(HW, AC + HW)
            r98 = (AC - 1) * AC2
            sqS, sqX = fA[:, r98 : r98 + HW], fA[:, r98 + AC : r98 + AC + HW]
            spS = fA[:, r98 + HW : r98 + 2 * HW]
            spX = fA[:, r98 + AC + HW : r98 + AC + 2 * HW]

            def s49(tag):
                return sml.tile([BS, HW], f32, tag=tag, name=tag)

            inq, inp_, t1, t2 = s49("inq"), s49("inp"), s49("t1"), s49("t2")
            aq, ap_ = s49("aq"), s49("ap")
            w1, w2, us, vs = s49("w1"), s49("w2"), s49("us"), s49("vs")
            kv, rkv, sq, sp = s49("kv"), s49("rkv"), s49("sq"), s49("sp")
            s2 = sml.tile([BS, 1], f32, tag="s2", name="s2")
            ebias = sml.tile([BS, 1], f32, tag="ebias", name="ebias")
            nc.vector.memset(ebias[:], EXP_BIAS)
            lg = sml.tile([BS, 1], f32, tag="lg", name="lg")
            lgf = sml.tile([BS, 1], f32, tag="lgf", name="lgf")

            # merged Gram blocks (G = S + X + X^T)
            qtpb = wrk.tile([BS, HW * HW], f32, tag="w", name="qtpb")
            ptqb = wrk.tile([BS, HW * HW], f32, tag="w", name="ptqb")

            def v3(t):  # [128, 49, 49] view of a [128, 2401] tile
                return t[:].rearrange("p (q c) -> p q c", c=HW)

            def v3t(t):  # transposed view (strides 1, 49)
                return t[:].rearrange("p (q c) -> p c q", c=HW)

            nc.vector.tensor_add(v3(qtpb), Sqp, Xqp)
            nc.vector.tensor_add(v3(qtpb), v3(qtpb), XpqT)
            nc.vector.tensor_add(v3(ptqb), Spq, Xpq)
            nc.vector.tensor_add(v3(ptqb), v3(ptqb), XqpT)
            nc.vector.tensor_add(sq[:], sqS, sqX)  # 1^T Q
            nc.vector.tensor_add(sp[:], spS, spX)  # 1^T P

            for (sx, dS, dX, inv) in (
                (sq, dqS, dqX, inq),
                (sp, dpS, dpX, inp_),
            ):
                # d = diag: dS + 2*dX ; u = d - s^2/C ; inv = rsqrt(u)
                nc.vector.scalar_tensor_tensor(
                    t2[:], dX, 2.0, dS, Alu.mult, Alu.add
                )
                nc.vector.tensor_mul(t1[:], sx[:], sx[:])
                nc.vector.scalar_tensor_tensor(
                    t2[:], t1[:], -1.0 / C, t2[:], Alu.mult, Alu.add
                )
                nc.scalar.activation(t1[:], t2[:], Act.Sqrt)
                nc.vector.reciprocal(inv[:], t1[:])
                # Newton polish for rsqrt: y = y*(1.5 - 0.5*u*y^2)
                nc.vector.tensor_mul(t1[:], inv[:], inv[:])
                nc.vector.tensor_mul(t1[:], t1[:], t2[:])
                nc.vector.tensor_scalar(t1[:], t1[:], -0.5, 1.5, Alu.mult, Alu.add)
                nc.vector.tensor_mul(inv[:], inv[:], t1[:])

            rC = 1.0 / np.sqrt(float(C))
            nc.vector.scalar_tensor_tensor(
                aq[:], sq[:], rC, inq[:], Alu.mult, Alu.mult
            )
            nc.vector.scalar_tensor_tensor(
                ap_[:], sp[:], rC, inp_[:], Alu.mult, Alu.mult
            )

            simb = big.tile([BS, HW * HW], f32, tag="sim", name="sim")
            Kb = big.tile([BS, HW * HW], f32, tag="K", name="K")
            Ktb = big.tile([BS, HW * HW], f32, tag="Kt", name="Kt")
            b1 = wrk.tile([BS, HW * HW], f32, tag="w", name="b1")
            b3 = wrk.tile([BS, HW * HW], f32, tag="w", name="b3")
            simTb = wrk.tile([BS, HW * HW], f32, tag="w", name="simTb")

            bq = inq[:].unsqueeze(2).broadcast_to([BS, HW, HW])
            bp = inp_[:].unsqueeze(1).broadcast_to([BS, HW, HW])
            nc.vector.tensor_mul(v3(b1), bq, bp)  # B1 = inq x inp
            nc.vector.tensor_mul(v3(simb), v3(qtpb), v3(b1))  # B2
            baq = aq[:].unsqueeze(2).broadcast_to([BS, HW, HW])
            bap = ap_[:].unsqueeze(1).broadcast_to([BS, HW, HW])
            nc.vector.tensor_mul(v3(b3), baq, bap)  # B3 = aq x ap
            nc.vector.tensor_sub(v3(simb), v3(simb), v3(b3))  # sim = B2 - B3
            # transposed side via transposed views of B1/B3
            nc.vector.tensor_mul(v3(simTb), v3(ptqb), v3t(b1))
            nc.vector.tensor_sub(v3(simTb), v3(simTb), v3t(b3))
            nc.scalar.activation(
                Kb[:], simb[:], Act.Exp, scale=1.0 / EPS_S, bias=ebias[:]
            )
            nc.scalar.activation(
                Ktb[:], simTb[:], Act.Exp, scale=1.0 / EPS_S, bias=ebias[:]
            )

            # weight vectors: w = relu(rowsum/49) + 0.001 (unnormalized; the
            # r-normalization cancels in the logits, the c-normalization is a
            # final 1/s2 scale)
            nc.vector.tensor_reduce(w1[:], v3(qtpb), axis=AxX, op=Alu.add)
            nc.vector.tensor_reduce(w2[:], v3(ptqb), axis=AxX, op=Alu.add)
            for w in (w1, w2):
                nc.vector.tensor_scalar(w[:], w[:], 1.0 / HW, 0.0, Alu.mult, Alu.max)
                nc.vector.tensor_scalar(w[:], w[:], 0.001, None, Alu.add)
            nc.vector.tensor_reduce(s2[:], w2[:], axis=AxX, op=Alu.add)

            # ---------------- Phase 2: Sinkhorn (Gauss-Seidel, linear) --------
            tb = wrk.tile([BS, HW * HW], f32, tag="w", name="tb")
            bvs = vs[:].unsqueeze(1).broadcast_to([BS, HW, HW])
            bus = us[:].unsqueeze(1).broadcast_to([BS, HW, HW])
            for it in range(ITERS):
                if it == 0:
                    nc.vector.tensor_reduce(kv[:], v3(Kb), axis=AxX, op=Alu.add)
                else:
                    nc.vector.tensor_mul(v3(tb), v3(Kb), bvs)
                    nc.vector.tensor_reduce(kv[:], v3(tb), axis=AxX, op=Alu.add)
                nc.vector.reciprocal(rkv[:], kv[:])
                nc.vector.tensor_mul(us[:], w1[:], rkv[:])
                nc.vector.tensor_mul(v3(tb), v3(Ktb), bus)
                nc.vector.tensor_reduce(kv[:], v3(tb), axis=AxX, op=Alu.add)
                nc.vector.reciprocal(rkv[:], kv[:])
                nc.vector.tensor_mul(vs[:], w2[:], rkv[:])

            # ---------------- Phase 3: logits ---------------------------------
            nc.vector.tensor_mul(v3(tb), v3(Kb), bvs)
            nc.vector.tensor_mul(tb[:], tb[:], simb[:])
            nc.vector.tensor_reduce(kv[:], v3(tb), axis=AxX, op=Alu.add)
            nc.vector.tensor_mul(kv[:], kv[:], us[:])
            nc.vector.tensor_reduce(lg[:], kv[:], axis=AxX, op=Alu.add)
            nc.vector.reciprocal(rkv[:, 0:1], s2[:])
            nc.vector.scalar_tensor_tensor(
                lgf[:], lg[:], TEMP, rkv[:, 0:1], Alu.mult, Alu.mult
            )  # (lg * T) / s2
            nc.sync.dma_start(outp[:, :], lgf[:])

    nc.compile()
    return nc


_NC = None


def _get_nc():
    global _NC
    if _NC is None:
        _NC = build_nc()
    return _NC


def _prep_in_maps(feature_map1, feature_map2):
    q = np.ascontiguousarray(np.asarray(feature_map1, dtype=np.float32)).reshape(
        B_FULL, C, HW
    )
    p = np.ascontiguousarray(np.asarray(feature_map2, dtype=np.float32)).reshape(
        B_FULL, C, HW
    )
    in_maps = []
    for i in range(NCORE):
        sl = slice(i * BS, (i + 1) * BS)
        a32 = np.empty((NCH, 128, BS, AC), np.float32)
        a32[..., AC - 1] = 1.0
        a32[..., 0:HW] = q[sl].reshape(BS, NCH, 128, HW).transpose(1, 2, 0, 3)
        a32[..., HW : 2 * HW] = p[sl].reshape(BS, NCH, 128, HW).transpose(1, 2, 0, 3)
        hi = a32.astype(ml_dtypes.bfloat16)
        lo = (a32 - hi.astype(np.float32)).astype(ml_dtypes.bfloat16)
        aug = np.empty((NCH, 128, BS, AC2), ml_dtypes.bfloat16)
        aug[..., 0:AC] = hi
        aug[..., AC:AC2] = lo
        # sequence DRAM as [group, channel-partition, chunk, batch, col] so
        # each per-group load DMA reads one contiguous span
        aug = np.ascontiguousarray(
            aug.reshape(NCH, 128, NGRP, GRP, AC2).transpose(2, 1, 0, 3, 4)
        ).reshape(NGRP, 128, NCH * GRP * AC2)
        in_maps.append({"aug": aug})
    return in_maps


def run(feature_map1, feature_map2, trace=False):
    in_maps = _prep_in_maps(feature_map1, feature_map2)
    nc = _get_nc()
    res = run_bass_kernel_spmd(nc, in_maps, core_ids=list(range(NCORE)), trace=trace)
    out = np.concatenate(
        [np.asarray(res.results[i]["out"]).reshape(BS) for i in range(NCORE)]
    ).astype(np.float32)
    return out, res


def kernel(feature_map1, feature_map2):
    out, _ = run(feature_map1, feature_map2, trace=False)
    return out


# revision 21
# speedup vs baseline: 1.0363x; 1.0363x over previous
"""DeepEMD Trainium2 kernel: batched 49x49 entropic-OT (Sinkhorn) similarity.

Strategy (8 NeuronCores, data-parallel over batch):
- Each core gets 128 batches. Host prepacks, per (chunk j of 128 channels,
  batch b), a matrix [Ahi | Alo] (128 x 198) where A = [Q | P | 1] (99 cols)
  split into bf16 hi + bf16 lo (near-lossless fp32 split), laid out so DMA
  loads are large contiguous runs.
- PE computes, per batch, [S | X] = Ahi^T [Ahi | Alo] (99 x 198, fp32 PSUM)
  with ONE accumulating bf16 matmul per channel chunk (weights widened to
  128 cols to engage fast-weight-load; junk rows ignored). The true Gram is
  G = S + X + X^T (the lo.lo term ~1e-5 is dropped); the X^T merge happens
  later as strided-view adds in the flat domain.
- A per-batch SBUF->SBUF DMA flattens [99, 198] into row b of a
  [128, 99*198] tile: everything after that runs batch-on-partitions with
  full 128-lane DVE. G contains Q^T P, P^T Q, column sums (ones row) and
  diag blocks -> similarity map, norms and weight vectors are cheap fixups.
- Sinkhorn runs in the *linear* domain (K = exp((sim-1)/eps + 16)) with
  Gauss-Seidel updates us = r/(K vs), vs = c/(K^T us). The reference's 100
  log-domain iterations are converged to ~1e-12 by 20; ITERS linear f32
  iterations reach ~2.5e-4 at 6 / ~1e-5 at 10.
- logits[b] = T * sum(flow * sim) = T * us^T ((K.sim) vs), two big DVE ops.
"""

import os
import sys

import numpy as np

sys.path.insert(0, "/opt/trn_rl_repo")

import concourse.bass as bass
import concourse.bacc as bacc
import concourse.mybir as mybir
from concourse import tile
from concourse.bass_utils import run_bass_kernel_spmd

import ml_dtypes

B_FULL, C, HW = 1024, 512, 49
NCORE = 8
BS = B_FULL // NCORE  # 128 batches per core
NCH = C // 128  # 4 chunks of 128 channels (PE contraction dim)
AC = 2 * HW + 1  # 99 augmented columns [Q | P | 1]
AC2 = 2 * AC  # 198 = [hi | lo]
GRP = 8  # batches per DMA group
NGRP = BS // GRP
ITERS = 6
EPS_S = 0.05
TEMP = 12.5 / HW
EXP_BIAS = -4.0  # exp((sim-1)/eps) * e^16 rescale; cancels in us*K*vs

f32 = mybir.dt.float32
bf16 = mybir.dt.bfloat16
Alu = mybir.AluOpType
Act = mybir.ActivationFunctionType
AxX = mybir.AxisListType.X


def build_nc(debug=False):
    nc = bacc.Bacc(None, target_bir_lowering=False, debug=debug)
    aug = nc.declare_dram_parameter(
        "aug", [NGRP, 128, NCH * GRP * AC2], bf16, isOutput=False
    )
    outp = nc.declare_dram_parameter("out", [BS, 1], f32, isOutput=True)

    FW = AC * AC2  # 19602 flat row width

    with tile.TileContext(nc) as tc:
        with (
            tc.tile_pool(name="big", bufs=1) as big,
            tc.tile_pool(name="stage", bufs=3) as stg,
            tc.tile_pool(name="gcopy", bufs=8) as gcp,
            tc.tile_pool(name="work", bufs=4) as wrk,
            tc.tile_pool(name="small", bufs=1) as sml,
            tc.tile_pool(name="psum", bufs=8, space="PSUM") as pp,
        ):
            flatG = big.tile([BS, FW], f32, tag="flatG", name="flatG")

            # ---------------- Phase 1: DMA in + Gram + flatten ----------------
            JW = GRP * AC2  # cols per chunk-slab
            for g in range(NGRP):
                th = stg.tile([128, NCH * JW], bf16, tag="h", name="hg")
                # host pre-sequenced: contiguous DRAM spans; split across the
                # two HWDGE rings (SP + ACT issue) for pipelining
                for j in range(NCH):
                    dmae = nc.sync if j % 2 == 0 else nc.scalar
                    dmae.dma_start(
                        th[:, j * JW : (j + 1) * JW],
                        aug[g, :, j * JW : (j + 1) * JW],
                    )
                for bb in range(GRP):
                    b = g * GRP + bb
                    ps = pp.tile([128, AC2], f32, tag="gram", name="gram")
                    for j in range(NCH):
                        base = j * JW + bb * AC2
                        # stationary: hi cols widened to 128 (spills into own
                        # lo region -> junk G rows 99..127, never read)
                        nc.tensor.matmul(
                            ps[:],
                            th[:, base : base + 128],
                            th[:, base : base + AC2],
                            start=(j == 0),
                            stop=(j == NCH - 1),
                        )
                    gs = gcp.tile([AC, AC2], f32, tag="gs", name="gs")
                    nc.vector.tensor_copy(gs[:], ps[0:AC, :])
                    # flatten [99, 198] -> one batch-major row; alternate the
                    # issuing HWDGE queue so neither sequencer becomes the
                    # phase-1 pacer
                    dmae = nc.sync if (b % 2 == 0) else nc.scalar
                    dmae.dma_start(flatG[b : b + 1, :], gs[:])

            # ---------------- Phase 1.5: fixup to sim/K/marginals -------------
            # flat row layout: element (m, n) of S at m*198+n, of X at
            # m*198+99+n, where S = hi^T hi, X = hi^T lo, G = S + X + X^T.
            fA = flatG[:]

            def mkview(row0, col0, dims):
                v = fA[:, row0 * AC2 + col0 : row0 * AC2 + col0 + 1].copy()
                v.ap = bass.mybir.VecI64Pair([list(v.ap[0])] + dims)
                return v

            def blk(row0, col0):
                # [128, 49, 49] strided view: (m, n) -> (row0+m)*198 + col0+n
                return mkview(row0, col0, [[AC2, HW], [1, HW]])

            def blkT(row0, col0):
                # transposed: out (m, n) reads (row0+n)*198 + col0+m
                return mkview(row0, col0, [[1, HW], [AC2, HW]])

            def dview(row0, col0):
                # [128, 49] diagonal view: (row0+m)*198 + col0+m, stride 199
                return mkview(row0, col0, [[AC2 + 1, HW]])

            Sqp, Xqp, XpqT = blk(0, HW), blk(0, AC + HW), blkT(HW, AC)
            Spq, Xpq, XqpT = blk(HW, 0), blk(HW, AC), blkT(0, AC + HW)
            dqS, dqX = dview(0, 0), dview(0, AC)
            dpS, dpX = dview(HW, HW), dview(HW, AC + HW)
            r98 = (AC - 1) * AC2
            sqS, sqX = fA[:, r98 : r98 + HW], fA[:, r98 + AC : r98 + AC + HW]
            spS = fA[:, r98 + HW : r98 + 2 * HW]
            spX = fA[:, r98 + AC + HW : r98 + AC + 2 * HW]

            def s49(tag):
                return sml.tile([BS, HW], f32, tag=tag, name=tag)

            inq, inp_, t1, t2 = s49("inq"), s49("inp"), s49("t1"), s49("t2")
            aq, ap_ = s49("aq"), s49("ap")
            w1, w2, us, vs = s49("w1"), s49("w2"), s49("us"), s49("vs")
            kv, rkv, sq, sp = s49("kv"), s49("rkv"), s49("sq"), s49("sp")
            s2 = sml.tile([BS, 1], f32, tag="s2", name="s2")
            ebias = sml.tile([BS, 1], f32, tag="ebias", name="ebias")
            nc.vector.memset(ebias[:], EXP_BIAS)
            lg = sml.tile([BS, 1], f32, tag="lg", name="lg")
            lgf = sml.tile([BS, 1], f32, tag="lgf", name="lgf")

            def v3(t):  # [128, 49, 49] view of a [128, 2401] tile
                return t[:].rearrange("p (q c) -> p q c", c=HW)

            def v3t(t):  # transposed view (strides 1, 49)
                return t[:].rearrange("p (q c) -> p c q", c=HW)

            # merged Gram blocks (G = S + X + X^T)
            qtpb = wrk.tile([BS, HW * HW], f32, tag="w", name="qtpb")
            ptqb = wrk.tile([BS, HW * HW], f32, tag="w", name="ptqb")

            nc.vector.tensor_add(v3(qtpb), Sqp, Xqp)
            nc.vector.tensor_add(v3(qtpb), v3(qtpb), XpqT)
            nc.vector.tensor_add(v3(ptqb), Spq, Xpq)
            nc.vector.tensor_add(v3(ptqb), v3(ptqb), XqpT)
            nc.vector.tensor_add(sq[:], sqS, sqX)  # 1^T Q
            nc.vector.tensor_add(sp[:], spS, spX)  # 1^T P

            # weight vectors early (qtpb/ptqb slots free sooner): w =
            # relu(rowsum/49) + 0.001, unnormalized (r-normalization cancels
            # in the logits; c-normalization is a final 1/s2 scale)
            nc.vector.tensor_reduce(w1[:], v3(qtpb), axis=AxX, op=Alu.add)
            nc.vector.tensor_reduce(w2[:], v3(ptqb), axis=AxX, op=Alu.add)
            for w in (w1, w2):
                nc.vector.tensor_scalar(w[:], w[:], 1.0 / HW, 0.0, Alu.mult, Alu.max)
                nc.vector.tensor_scalar(w[:], w[:], 0.001, None, Alu.add)
            nc.vector.tensor_reduce(s2[:], w2[:], axis=AxX, op=Alu.add)

            for (sx, dS, dX, inv) in (
                (sq, dqS, dqX, inq),
                (sp, dpS, dpX, inp_),
            ):
                # d = diag: dS + 2*dX ; u = d - s^2/C ; inv = rsqrt(u)
                nc.vector.scalar_tensor_tensor(
                    t2[:], dX, 2.0, dS, Alu.mult, Alu.add
                )
                nc.vector.tensor_mul(t1[:], sx[:], sx[:])
                nc.vector.scalar_tensor_tensor(
                    t2[:], t1[:], -1.0 / C, t2[:], Alu.mult, Alu.add
                )
                nc.scalar.activation(t1[:], t2[:], Act.Sqrt)
                nc.vector.reciprocal(inv[:], t1[:])
                # Newton polish for rsqrt: y = y*(1.5 - 0.5*u*y^2)
                nc.vector.tensor_mul(t1[:], inv[:], inv[:])
                nc.vector.tensor_mul(t1[:], t1[:], t2[:])
                nc.vector.tensor_scalar(t1[:], t1[:], -0.5, 1.5, Alu.mult, Alu.add)
                nc.vector.tensor_mul(inv[:], inv[:], t1[:])

            rC = 1.0 / np.sqrt(float(C))
            nc.vector.scalar_tensor_tensor(
                aq[:], sq[:], rC, inq[:], Alu.mult, Alu.mult
            )
            nc.vector.scalar_tensor_tensor(
                ap_[:], sp[:], rC, inp_[:], Alu.mult, Alu.mult
            )

            simb = big.tile([BS, HW * HW], f32, tag="sim", name="sim")
            Kb = big.tile([BS, HW * HW], f32, tag="K", name="K")
            Ktb = big.tile([BS, HW * HW], f32, tag="Kt", name="Kt")
            b1 = wrk.tile([BS, HW * HW], f32, tag="w", name="b1")
            b3 = wrk.tile([BS, HW * HW], f32, tag="w", name="b3")
            simTb = wrk.tile([BS, HW * HW], f32, tag="w", name="simTb")

            bq = inq[:].unsqueeze(2).broadcast_to([BS, HW, HW])
            bp = inp_[:].unsqueeze(1).broadcast_to([BS, HW, HW])
            nc.vector.tensor_mul(v3(b1), bq, bp)  # B1 = inq x inp
            nc.vector.tensor_mul(v3(simb), v3(qtpb), v3(b1))  # B2
            baq = aq[:].unsqueeze(2).broadcast_to([BS, HW, HW])
            bap = ap_[:].unsqueeze(1).broadcast_to([BS, HW, HW])
            nc.vector.tensor_mul(v3(b3), baq, bap)  # B3 = aq x ap
            nc.vector.tensor_sub(v3(simb), v3(simb), v3(b3))  # sim = B2 - B3
            # transposed side via transposed views of B1/B3
            nc.vector.tensor_mul(v3(simTb), v3(ptqb), v3t(b1))
            nc.vector.tensor_sub(v3(simTb), v3(simTb), v3t(b3))
            nc.scalar.activation(
                Kb[:], simb[:], Act.Exp, scale=1.0 / EPS_S, bias=ebias[:]
            )
            nc.scalar.activation(
                Ktb[:], simTb[:], Act.Exp, scale=1.0 / EPS_S, bias=ebias[:]
            )


            # ---------------- Phase 2: Sinkhorn (Gauss-Seidel, linear) --------
            tb = wrk.tile([BS, HW * HW], f32, tag="w", name="tb")
            bvs = vs[:].unsqueeze(1).broadcast_to([BS, HW, HW])
            bus = us[:].unsqueeze(1).broadcast_to([BS, HW, HW])
            for it in range(ITERS):
                if it == 0:
                    nc.vector.tensor_reduce(kv[:], v3(Kb), axis=AxX, op=Alu.add)
                else:
                    nc.vector.tensor_mul(v3(tb), v3(Kb), bvs)
                    nc.vector.tensor_reduce(kv[:], v3(tb), axis=AxX, op=Alu.add)
                nc.vector.reciprocal(rkv[:], kv[:])
                nc.vector.tensor_mul(us[:], w1[:], rkv[:])
                nc.vector.tensor_mul(v3(tb), v3(Ktb), bus)
                nc.vector.tensor_reduce(kv[:], v3(tb), axis=AxX, op=Alu.add)
                nc.vector.reciprocal(rkv[:], kv[:])
                nc.vector.tensor_mul(vs[:], w2[:], rkv[:])

            # ---------------- Phase 3: logits ---------------------------------
            nc.vector.tensor_mul(v3(tb), v3(Kb), bvs)
            nc.vector.tensor_mul(tb[:], tb[:], simb[:])
            nc.vector.tensor_reduce(kv[:], v3(tb), axis=AxX, op=Alu.add)
            nc.vector.tensor_mul(kv[:], kv[:], us[:])
            nc.vector.tensor_reduce(lg[:], kv[:], axis=AxX, op=Alu.add)
            nc.vector.reciprocal(rkv[:, 0:1], s2[:])
            nc.vector.scalar_tensor_tensor(
                lgf[:], lg[:], TEMP, rkv[:, 0:1], Alu.mult, Alu.mult
            )  # (lg * T) / s2
            nc.sync.dma_start(outp[:, :], lgf[:])

    nc.compile()
    return nc


_NC = None


def _get_nc():
    global _NC
    if _NC is None:
        _NC = build_nc()
    return _NC


def _prep_in_maps(feature_map1, feature_map2):
    q = np.ascontiguousarray(np.asarray(feature_map1, dtype=np.float32)).reshape(
        B_FULL, C, HW
    )
    p = np.ascontiguousarray(np.asarray(feature_map2, dtype=np.float32)).reshape(
        B_FULL, C, HW
    )
    in_maps = []
    for i in range(NCORE):
        sl = slice(i * BS, (i + 1) * BS)
        a32 = np.empty((NCH, 128, BS, AC), np.float32)
        a32[..., AC - 1] = 1.0
        a32[..., 0:HW] = q[sl].reshape(BS, NCH, 128, HW).transpose(1, 2, 0, 3)
        a32[..., HW : 2 * HW] = p[sl].reshape(BS, NCH, 128, HW).transpose(1, 2, 0, 3)
        hi = a32.astype(ml_dtypes.bfloat16)
        lo = (a32 - hi.astype(np.float32)).astype(ml_dtypes.bfloat16)
        aug = np.empty((NCH, 128, BS, AC2), ml_dtypes.bfloat16)
        aug[..., 0:AC] = hi
        aug[..., AC:AC2] = lo
        # sequence DRAM as [group, channel-partition, chunk, batch, col] so
        # each per-group load DMA reads one contiguous span
        aug = np.ascontiguousarray(
            aug.reshape(NCH, 128, NGRP, GRP, AC2).transpose(2, 1, 0, 3, 4)
        ).reshape(NGRP, 128, NCH * GRP * AC2)
        in_maps.append({"aug": aug})
    return in_maps


def run(feature_map1, feature_map2, trace=False):
    in_maps = _prep_in_maps(feature_map1, feature_map2)
    nc = _get_nc()
    res = run_bass_kernel_spmd(nc, in_maps, core_ids=list(range(NCORE)), trace=trace)
    out = np.concatenate(
        [np.asarray(res.results[i]["out"]).reshape(BS) for i in range(NCORE)]
    ).astype(np.float32)
    return out, res


def kernel(feature_map1, feature_map2):
    out, _ = run(feature_map1, feature_map2, trace=False)
    return out
